# revision 12
# baseline (speedup 1.0000x reference)
"""Trainium2 Bass kernel for 2-layer single-head GAT (nn_GAT__80942953660642).

Strategy (8 NeuronCores, SPMD), v2 — dma_gather edge phase:
  - Nodes sharded contiguously: core c owns nodes [c*12500, (c+1)*12500).
  - Host assigns each core's 12500 dsts a RANK permutation (shape-packed, see
    below) and pre-permutes X rows into rank order. All device tables are in
    rank space: global row of (core c, rank i) = c*12544 + i.
  - Phase M: h = Xperm_shard @ W0aug on PE. W0aug = [W0, W0@al0, W0@ar0], so
    the PSUM tile [128, 142] directly holds h(140), el, er. Rows are written
    fp16 into a 256-half (512B) table row; el/er columns are also saved to
    SBUF (el_all/er_all) for the local self-edge term.
  - AllGather the 6.4MB fp16 shard tables -> full 51.4MB table0 per core.
  - Phase E0: per core, dsts are packed into 98 groups of 128 (one dst per
    partition). Non-self in-edges of each dst occupy slot blocks in a
    [128, nblocks, 256] fp16 SBUF grid, fetched with nc.gpsimd.dma_gather:
    one call per (chunk-of-groups, 32768-row table region window), int16
    region-relative indices (wrapped in 16 partitions, replicated x8).
    Slots are banded by region; per-(group,region) band widths are shared
    across cores (max), padding slots point at sentinel rows whose el=-30000
    kills them in the softmax. Self edge is computed from el_all/er_all and a
    contiguous DMA of the group's own table rows (no gather).
  - Edge softmax with constant -5 logit bias (fp16-safe, cancels in the
    ratio); fp16 DVE multiply-add accumulation over slot blocks.
  - hp1/el1/er1 = h1 @ W1aug via PE transpose + matmul; fp32 64-float rows
    form table1 (AllGather #2); Phase E1 repeats the edge phase at width 7
    reusing the SAME index buffers (tables share rank-space numbering).
  - Host scatters the rank-ordered [12544, 7] outputs back to node order.
"""
import sys
sys.path.insert(0, "/opt/trn_rl_repo")
import numpy as np

N = 100000
NCORES = 8
SHARD = 12500
PSHARD = 12544          # 98 * 128
G = PSHARD // 128       # 98 groups
REG = 32768             # int16 index window (table region)
NREG = 4
KDIM = 1536             # 1433 padded to 12*128
D0 = 140
D0A = 142               # h + el + er
D1 = 7
E0W = 256               # fp16 halves per L0 table row (512B)
E1W = 64                # fp32 floats per L1 table row (256B)
SENT = np.float32(-30000.0)
BLK0 = 96               # max slot blocks per E0 gather chunk (48KB/partition)

_CACHE = {}
TRACE = False
LAST_EXEC_NS = None


def _regions_of_rows(rows):
    return rows // REG


def _host_prep(src, dst):
    """Pack dsts into groups, build shared band schedule + int16 idx buffers."""
    src = np.asarray(src).astype(np.int64)
    dst = np.asarray(dst).astype(np.int64)
    nodes = np.arange(N, dtype=np.int64)

    # --- pass 1: per-core region counts (identity ranks fix src regions) ---
    id_row = (nodes // SHARD) * PSHARD + (nodes % SHARD)   # identity rank row
    src_reg_id = _regions_of_rows(id_row[src])

    percore = []
    for c in range(NCORES):
        lo = c * SHARD
        m = (dst >= lo) & (dst < lo + SHARD)
        e_dst = dst[m] - lo
        e_src = src[m]
        nself = e_src != (e_dst + lo)
        cnt = np.zeros((SHARD, NREG), np.int64)
        np.add.at(cnt, (e_dst[nself], src_reg_id[m][nself]), 1)
        percore.append(dict(e_dst=e_dst[nself], e_src=e_src[nself],
                            sreg=src_reg_id[m][nself], cnt=cnt))

    # --- pass 2: per-core rank permutation within identity region-sides ---
    # A core's rank range can straddle one 32768 boundary; keep each dst on
    # its identity side so every node's region never changes.
    for c in range(NCORES):
        pc = percore[c]
        cnt = pc["cnt"]
        base = c * PSHARD
        split = None
        for r in range(1, NREG):
            b = r * REG - base
            if 0 < b < SHARD:
                split = b
        sides = [np.arange(SHARD)] if split is None else \
            [np.arange(split), np.arange(split, SHARD)]
        rank_of = np.empty(SHARD, np.int64)
        pos = 0
        for ids in sides:
            cc = cnt[ids]
            s = -np.sort(-cc, axis=1)   # shape key: sorted counts desc
            order = np.lexsort((-cc[:, 0], -s[:, 3], -s[:, 2], -s[:, 1], -s[:, 0]))
            rank_of[ids[order]] = pos + np.arange(len(ids))
            pos += len(ids)
        pc["rank_of"] = rank_of          # local dst id -> rank (0..12499)
        node_of = np.full(PSHARD, -1, np.int64)
        node_of[rank_of] = np.arange(SHARD)
        pc["node_of_rank"] = node_of     # rank -> local dst id (-1 = dummy)

    # global rank-space row of every node
    rank_row = np.empty(N, np.int64)
    for c in range(NCORES):
        rank_row[c * SHARD + np.arange(SHARD)] = c * PSHARD + percore[c]["rank_of"]

    # --- band schedule (shared across cores) ---
    Ball = np.zeros((NCORES, G, NREG), np.int64)
    for c in range(NCORES):
        pc = percore[c]
        cnt_rank = np.zeros((PSHARD, NREG), np.int64)
        cnt_rank[pc["rank_of"]] = pc["cnt"]
        Ball[c] = cnt_rank.reshape(G, 128, NREG).max(axis=1)
        pc["cnt_rank"] = cnt_rank
    B = Ball.max(axis=0)                 # [G, NREG] shared band widths
    Kg = B.sum(axis=1)                   # blocks per group

    # --- chunking: E0 chunks bounded by BLK0 blocks; E1 = pairs of E0 chunks
    chunks = []
    gs = 0
    while gs < G:
        ge = gs + 1
        while ge < G and Kg[gs:ge + 1].sum() <= BLK0 and ge - gs < 8:
            ge += 1
        chunks.append((gs, ge))
        gs = ge
    e1chunks = [(chunks[i][0], chunks[min(i + 1, len(chunks) - 1)][1])
                for i in range(0, len(chunks), 2)]

    # per-(chunk) block layout: region-major, then group, then band blocks
    # block offset tables (shared): for group g, list of (region, start_in_chunk)
    chunk_layout = []
    for (gs, ge) in chunks:
        off = 0
        calls = []            # (region, blkstart, nblocks, [(g, bandwidth)...])
        gblocks = {g: [] for g in range(gs, ge)}
        for r in range(NREG):
            nb = int(B[gs:ge, r].sum())
            if nb == 0:
                continue
            s = off
            for g in range(gs, ge):
                for j in range(int(B[g, r])):
                    gblocks[g].append(off)
                    off += 1
            calls.append((r, s, nb))
        chunk_layout.append(dict(gs=gs, ge=ge, nblk=off, calls=calls,
                                 gblocks=gblocks))

    # sentinel row per region (dummy ranks 12500.. of a core in that region)
    sent_rows = {}
    for c in range(NCORES):
        rows = c * PSHARD + np.arange(SHARD, PSHARD)
        for row in rows:
            sent_rows.setdefault(int(row // REG), int(row))
    assert set(sent_rows) == set(range(NREG))

    # --- idx buffers (per core), shared E0/E1, col-major per chunk/region ---
    idx_cols = sum(cl["nblk"] * 8 for cl in chunk_layout)
    for c in range(NCORES):
        pc = percore[c]
        # slot lists per (dst rank, region): rank-space src rows
        er = pc["e_dst"]
        rr = pc["rank_of"][er]           # dst rank per edge
        srow = rank_row[pc["e_src"]]     # global rank row of src
        sreg = pc["sreg"]
        order = np.lexsort((srow, sreg, rr))
        rr, srow, sreg = rr[order], srow[order], sreg[order]
        # position of edge within its (rank, region) run
        key = rr * NREG + sreg
        first = np.searchsorted(key, key)
        kpos = np.arange(len(key)) - first
        slot = np.full((PSHARD, NREG, int(B.max())), -1, np.int64)
        slot[rr, sreg, kpos] = srow
        idxbuf = np.empty((128, idx_cols), np.int16)
        col = 0
        for cl in chunk_layout:
            for (r, s, nb) in cl["calls"]:
                vals = np.empty((nb, 128), np.int64)   # block-major
                bi = 0
                for g in range(cl["gs"], cl["ge"]):
                    for j in range(int(B[g, r])):
                        v = slot[g * 128:(g + 1) * 128, r, j]
                        v = np.where(v < 0, sent_rows[r], v)
                        vals[bi] = v - r * REG
                        bi += 1
                flat = vals.reshape(-1)                # i = blk*128 + p
                w = np.ascontiguousarray(
                    flat.reshape(-1, 16).T).astype(np.int16)  # [16, n/16]
                ncols = w.shape[1]
                idxbuf[:, col:col + ncols] = np.tile(w, (8, 1))
                col += ncols
        assert col == idx_cols
        pc["idxbuf"] = idxbuf
    return percore, B, chunks, e1chunks, chunk_layout, idx_cols


def _prep_weights(inputs, percore):
    X = np.asarray(inputs["X"], np.float32)
    W0 = np.asarray(inputs["W0"], np.float32)
    al0 = np.asarray(inputs["al0"], np.float32)
    ar0 = np.asarray(inputs["ar0"], np.float32)
    b0 = np.asarray(inputs["b0"], np.float32)
    W1 = np.asarray(inputs["W1"], np.float32)
    al1 = np.asarray(inputs["al1"], np.float32)
    ar1 = np.asarray(inputs["ar1"], np.float32)
    b1 = np.asarray(inputs["b1"], np.float32)

    W0aug = np.concatenate([W0, (W0 @ al0)[:, None], (W0 @ ar0)[:, None]], 1)
    W0p = np.zeros((KDIM, D0A), np.float32)
    W0p[:1433] = W0aug
    W0r = np.ascontiguousarray(
        W0p.reshape(12, 128, D0A).transpose(1, 0, 2).reshape(128, 12 * D0A))
    W1aug = np.concatenate([W1, (W1 @ al1)[:, None], (W1 @ ar1)[:, None]], 1)
    W1a = np.zeros((128, D1 + 2), np.float16)
    W1a[:128] = W1aug[:128].astype(np.float16)
    W1b = np.zeros((128, D1 + 2), np.float16)
    W1b[:12] = W1aug[128:140].astype(np.float16)
    bc = lambda v, w, dt: np.broadcast_to(
        np.asarray(v, dt)[None, :], (128, w)).copy()
    identf = np.eye(128, dtype=np.float16)
    sentm = np.zeros((128, 1), np.float16)
    sentm[SHARD - (G - 1) * 128:, 0] = SENT
    sentm1 = np.zeros((128, 1), np.float32)
    sentm1[SHARD - (G - 1) * 128:, 0] = SENT
    com = dict(w0r=W0r, w1a=W1a, w1b=W1b, identf=identf, sentm=sentm,
               sentm1=sentm1, b0f=bc(b0, D0, np.float16),
               b1f=bc(b1, D1, np.float32))

    xts = []
    for c in range(NCORES):
        node_of = percore[c]["node_of_rank"]
        lo = c * SHARD
        Xp = np.zeros((PSHARD, KDIM), np.float32)
        real = node_of >= 0
        Xp[np.where(real)[0], :1433] = X[lo + node_of[real], :]
        xt = Xp.reshape(G, 128, 12, 128).transpose(0, 3, 2, 1).reshape(G, 128, 12 * 128)
        xt = np.concatenate([xt, np.zeros((2, 128, 12 * 128), np.float32)])
        xt = np.ascontiguousarray(
            xt.reshape(50, 2, 128, 1536).transpose(0, 2, 1, 3).reshape(50, 128, 2 * 1536))
        xts.append(xt)
    return com, xts


def _build(B, chunks, e1chunks, chunk_layout, idx_cols):
    import concourse.bass as bass
    import concourse.tile as tile
    from concourse import bacc, mybir, library_config
    dt = mybir.dt
    op = mybir.AluOpType
    act = mybir.ActivationFunctionType

    nc = bacc.Bacc("TRN2", target_bir_lowering=False, debug=False,
                   num_devices=NCORES)
    t_x = nc.dram_tensor("x_up", [50, 128, 2 * 1536], dt.float32, kind="ExternalInput")
    t_w0 = nc.dram_tensor("w0r", [128, 12 * D0A], dt.float32, kind="ExternalInput")
    t_w1a = nc.dram_tensor("w1a", [128, D1 + 2], dt.float16, kind="ExternalInput")
    t_w1b = nc.dram_tensor("w1b", [128, D1 + 2], dt.float16, kind="ExternalInput")
    t_b0 = nc.dram_tensor("b0f", [128, D0], dt.float16, kind="ExternalInput")
    t_b1 = nc.dram_tensor("b1f", [128, D1], dt.float32, kind="ExternalInput")
    t_id = nc.dram_tensor("identf", [128, 128], dt.float16, kind="ExternalInput")
    t_sm = nc.dram_tensor("sentm", [128, 1], dt.float16, kind="ExternalInput")
    t_sm1 = nc.dram_tensor("sentm1", [128, 1], dt.float32, kind="ExternalInput")
    t_idx = nc.dram_tensor("idxbuf", [128, idx_cols], dt.int16, kind="ExternalInput")
    t_out = nc.dram_tensor("out_buf", [PSHARD, D1], dt.float32, kind="ExternalOutput")

    NB = [cl["nblk"] for cl in chunk_layout]

    with tile.TileContext(nc) as tc:
        with tc.tile_pool(name="const", bufs=1) as cpool, \
             tc.tile_pool(name="xload", bufs=2) as xpool, \
             tc.tile_pool(name="idxp", bufs=2) as ipool, \
             tc.tile_pool(name="gath", bufs=2) as gpool, \
             tc.tile_pool(name="hex", bufs=3) as hexpool, \
             tc.tile_pool(name="work", bufs=3) as wpool, \
             tc.tile_pool(name="small", bufs=4) as spool, \
             tc.tile_pool(name="psum", bufs=2, space="PSUM") as ppool, \
             tc.tile_pool(name="dram", bufs=1, space="DRAM") as dpool:

            nc.gpsimd.load_library(library_config.mlp)

            w0_sb = cpool.tile([128, 12 * D0A], dt.float32)
            nc.sync.dma_start(w0_sb[:], t_w0[:])
            w1a_sb = cpool.tile([128, D1 + 2], dt.float16)
            nc.sync.dma_start(w1a_sb[:], t_w1a[:])
            w1b_sb = cpool.tile([128, D1 + 2], dt.float16)
            nc.sync.dma_start(w1b_sb[:], t_w1b[:])
            b0_sb = cpool.tile([128, D0], dt.float16)
            nc.sync.dma_start(b0_sb[:], t_b0[:])
            b1_sb = cpool.tile([128, D1], dt.float32)
            nc.sync.dma_start(b1_sb[:], t_b1[:])
            id_sb = cpool.tile([128, 128], dt.float16)
            nc.sync.dma_start(id_sb[:], t_id[:])
            sm_sb = cpool.tile([128, 1], dt.float16)
            nc.sync.dma_start(sm_sb[:], t_sm[:])
            sm1_sb = cpool.tile([128, 1], dt.float32)
            nc.sync.dma_start(sm1_sb[:], t_sm1[:])
            bm5 = cpool.tile([128, 1], dt.float32)
            nc.vector.memset(bm5[:], -5.0)
            el_all = cpool.tile([128, G], dt.float32)
            er_all = cpool.tile([128, G], dt.float32)
            el1_all = cpool.tile([128, G], dt.float32)
            er1_all = cpool.tile([128, G], dt.float32)

            shard0 = dpool.tile([PSHARD, E0W], dt.float16)
            table0 = dpool.tile([NCORES * PSHARD, E0W], dt.float16,
                                addr_space="Shared")
            shard1 = dpool.tile([PSHARD, E1W], dt.float32)
            table1 = dpool.tile([NCORES * PSHARD, E1W], dt.float32,
                                addr_space="Shared")

            # ---- Phase M ----
            xts = {}
            for n in range(G):
                b, t = n // 2, n % 2
                if t == 0:
                    xtile = xpool.tile([128, 2 * 1536], dt.float32, tag="xt")
                    xts[b] = xtile
                    nc.sync.dma_start(xtile[:], t_x[:][b])
                xt = xts[b][:, t * 1536:(t + 1) * 1536]
                ph = ppool.tile([128, D0A], dt.float32, space="PSUM", tag="ph")
                for k in range(12):
                    nc.tensor.matmul(ph[:], xt[:, k * 128:(k + 1) * 128],
                                     w0_sb[:, k * D0A:(k + 1) * D0A],
                                     start=(k == 0), stop=(k == 11))
                hx = hexpool.tile([128, E0W], dt.float16, tag="hex0")
                nc.vector.tensor_copy(hx[:, 0:D0A], ph[:])
                nc.vector.tensor_copy(el_all[:, n:n + 1], hx[:, 140:141])
                nc.vector.tensor_copy(er_all[:, n:n + 1], hx[:, 141:142])
                if n == G - 1:
                    nc.vector.tensor_tensor(hx[:, 140:141], hx[:, 140:141],
                                            sm_sb[:], op=op.add)
                nc.sync.dma_start(
                    shard0[:].rearrange("(g p) w -> g p w", p=128)[n], hx[:])

            nc.gpsimd.collective_compute(
                "AllGather", op.bypass, replica_groups=[list(range(NCORES))],
                ins=[shard0[:]], outs=[table0[:]])

            # ---- Phase E0 ----
            tab0v = table0[:]
            idx_col0 = {}
            col = 0
            for ci, cl in enumerate(chunk_layout):
                idx_col0[ci] = col
                col += cl["nblk"] * 8

            def edge_phase(layer):
                W = E0W if layer == 0 else E1W
                fdt = dt.float16 if layer == 0 else dt.float32
                tabv = tab0v if layer == 0 else table1[:]
                shardv = shard0[:] if layer == 0 else shard1[:]
                elA = el_all if layer == 0 else el1_all
                erA = er_all if layer == 0 else er1_all
                DD = D0 if layer == 0 else D1
                ELC, ERC = (140, 141) if layer == 0 else (7, 8)
                ch_list = ([(ci, cl) for ci, cl in enumerate(chunk_layout)]
                           if layer == 0 else None)
                if layer == 1:
                    # pairs of E0 chunks
                    ch_list = []
                    for i in range(0, len(chunk_layout), 2):
                        ch_list.append((i, None))
                for item in ch_list:
                    if layer == 0:
                        ci, cl = item
                        subs = [cl]
                        nblk = cl["nblk"]
                        colbase = idx_col0[ci]
                    else:
                        ci = item[0]
                        subs = chunk_layout[ci:ci + 2]
                        nblk = sum(s["nblk"] for s in subs)
                        colbase = idx_col0[ci]
                    icols = nblk * 8
                    idx_sb = ipool.tile([128, icols], dt.int16, tag=f"ix{layer}")
                    nc.sync.dma_start(idx_sb[:], t_idx[:, colbase:colbase + icols])
                    gt = gpool.tile([128, nblk * W], fdt, tag="g")
                    gv = gt[:].rearrange("p (k w) -> p k w", w=W)
                    boff = 0
                    coff = 0
                    for sub in subs:
                        for (r, s, nb) in sub["calls"]:
                            lo = r * REG
                            hi = min(lo + REG, NCORES * PSHARD)
                            nc.gpsimd.dma_gather(
                                gv[:, boff + s:boff + s + nb], tabv[lo:hi],
                                idx_sb[:, coff:coff + nb * 8],
                                nb * 128, nb * 128, W, single_packet=False)
                            coff += nb * 8
                        boff += sub["nblk"]
                    boff = 0
                    for sub in subs:
                        for g in range(sub["gs"], sub["ge"]):
                            blocks = [boff + x for x in sub["gblocks"][g]]
                            K = len(blocks)
                            ep = spool.tile([128, K + 1], fdt, tag=f"ep{layer}")
                            nc.vector.tensor_tensor(
                                ep[:, 0:1], elA[:, g:g + 1], erA[:, g:g + 1],
                                op=op.add)
                            o = 1
                            for (r, s, nb) in sub["calls"]:
                                gb = [x for x in sub["gblocks"][g]
                                      if s <= x < s + nb]
                                if not gb:
                                    continue
                                bw = len(gb)
                                elv = gv[:, boff + gb[0]:boff + gb[0] + bw,
                                         ELC:ELC + 1]
                                nc.vector.tensor_scalar(
                                    ep[:, o:o + bw],
                                    elv.rearrange("p b one -> p (b one)"),
                                    erA[:, g:g + 1], None, op.add)
                                o += bw
                            ee = spool.tile([128, K + 1], fdt, tag=f"ee{layer}")
                            nc.vector.scalar_tensor_tensor(
                                out=ee[:], in0=ep[:], scalar=0.2, in1=ep[:],
                                op0=op.mult, op1=op.max)
                            ex = spool.tile([128, K + 1], dt.float32,
                                            tag=f"ex{layer}")
                            dn = spool.tile([128, 1], dt.float32, tag=f"dn{layer}")
                            nc.scalar.activation(ex[:], ee[:], act.Exp,
                                                 bias=bm5[:], accum_out=dn[:])
                            nc.vector.tensor_scalar_max(dn[:], dn[:], 1e-30)
                            rv = spool.tile([128, 1], dt.float32, tag=f"rv{layer}")
                            nc.vector.reciprocal(rv[:], dn[:])
                            an = spool.tile([128, K + 1], dt.float32,
                                            tag=f"an{layer}")
                            nc.vector.tensor_scalar(an[:], ex[:], rv[:], None,
                                                    op.mult)
                            hs = hexpool.tile([128, W], fdt, tag=f"hs{layer}")
                            nc.sync.dma_start(
                                hs[:],
                                shardv.rearrange("(g p) w -> g p w", p=128)[g])
                            acc = wpool.tile([128, DD], fdt, tag=f"acc{layer}")
                            nc.vector.tensor_scalar(acc[:], hs[:, 0:DD],
                                                    an[:, 0:1], None, op.mult)
                            for k, bk in enumerate(blocks):
                                nc.vector.scalar_tensor_tensor(
                                    out=acc[:], in0=gv[:, bk, 0:DD],
                                    scalar=an[:, k + 1:k + 2], in1=acc[:],
                                    op0=op.mult, op1=op.add)
                            if layer == 0:
                                h1 = wpool.tile([128, D0], dt.float16, tag="h1")
                                nc.vector.tensor_tensor(h1[:], acc[:], b0_sb[:],
                                                        op=op.add)
                                nc.scalar.activation(h1[:], h1[:], act.Relu)
                                pt1 = ppool.tile([128, 128], dt.float16,
                                                 space="PSUM", tag="pt1")
                                nc.tensor.transpose(pt1[:], h1[:, 0:128], id_sb[:])
                                pt2 = ppool.tile([128, 128], dt.float16,
                                                 space="PSUM", tag="pt2")
                                nc.tensor.transpose(pt2[0:12, :], h1[:, 128:140],
                                                    id_sb[:])
                                t1s = wpool.tile([128, 128], dt.float16, tag="t1s")
                                nc.scalar.activation(t1s[:], pt1[:], act.Copy)
                                t2s = wpool.tile([128, 128], dt.float16, tag="t2s")
                                nc.scalar.activation(t2s[0:12, :], pt2[0:12, :],
                                                     act.Copy)
                                php = ppool.tile([128, D1 + 2], dt.float32,
                                                 space="PSUM", tag="php")
                                nc.tensor.matmul(php[:], t1s[:], w1a_sb[:],
                                                 start=True, stop=False)
                                nc.tensor.matmul(php[:], t2s[0:12, :],
                                                 w1b_sb[0:12, :],
                                                 start=False, stop=True)
                                hx1 = wpool.tile([128, E1W], dt.float32,
                                                 tag="hx1")
                                nc.vector.tensor_copy(hx1[:, 0:D1 + 2], php[:])
                                nc.vector.tensor_copy(el1_all[:, g:g + 1],
                                                      hx1[:, 7:8])
                                nc.vector.tensor_copy(er1_all[:, g:g + 1],
                                                      hx1[:, 8:9])
                                if g == G - 1:
                                    nc.vector.tensor_tensor(
                                        hx1[:, 7:8], hx1[:, 7:8], sm1_sb[:],
                                        op=op.add)
                                nc.sync.dma_start(
                                    shard1[:].rearrange("(g p) w -> g p w",
                                                        p=128)[g], hx1[:])
                            else:
                                ot = spool.tile([128, D1], dt.float32, tag="ot")
                                nc.vector.tensor_tensor(ot[:], acc[:], b1_sb[:],
                                                        op=op.add)
                                nc.scalar.activation(ot[:], ot[:], act.Relu)
                                nc.sync.dma_start(
                                    t_out[:].rearrange("(g p) w -> g p w",
                                                       p=128)[g], ot[:])
                        boff += sub["nblk"]

            edge_phase(0)
            nc.gpsimd.collective_compute(
                "AllGather", op.bypass, replica_groups=[list(range(NCORES))],
                ins=[shard1[:]], outs=[table1[:]])
            edge_phase(1)
    nc.compile()
    return nc


def kernel(**inputs):
    percore, B, chunks, e1chunks, chunk_layout, idx_cols = _host_prep(
        inputs["src"], inputs["dst"])
    com, xts = _prep_weights(inputs, percore)

    key = (tuple(B.reshape(-1)), idx_cols)
    if key not in _CACHE:
        _CACHE[key] = _build(B, chunks, e1chunks, chunk_layout, idx_cols)
    nc = _CACHE[key]

    in_maps = []
    for c in range(NCORES):
        m = dict(x_up=xts[c], idxbuf=percore[c]["idxbuf"], **{
            k: com[k] for k in ("w0r", "w1a", "w1b", "b0f", "b1f",
                                "identf", "sentm", "sentm1")})
        in_maps.append(m)

    from concourse.bass_utils import run_bass_kernel_spmd
    global LAST_EXEC_NS
    res = run_bass_kernel_spmd(nc, in_maps, core_ids=list(range(NCORES)),
                               trace=TRACE)
    LAST_EXEC_NS = res.exec_time_ns
    out = np.zeros((N, D1), dtype=np.float32)
    for c in range(NCORES):
        ob = res.results[c]["out_buf"]
        node_of = percore[c]["node_of_rank"]
        real = node_of >= 0
        out[c * SHARD + node_of[real]] = ob[np.where(real)[0]]
    return out


# revision 17
# speedup vs baseline: 1.4727x; 1.4727x over previous
"""Trainium2 Bass kernel for 2-layer single-head GAT (nn_GAT__80942953660642).

Strategy (8 NeuronCores, SPMD):
  - Nodes sharded contiguously: core c owns nodes [c*12500, (c+1)*12500).
  - Phase M: h = X_shard @ W0 on PE (host-pretransposed X tiles), el/er via
    DVE reduces; rows [h(140), el, er, pad] packed into a 144-f32 shard table.
  - AllGather the 7.2MB shard tables -> full 57.8MB node table per core.
  - Phase E0 (edge phase): per core, its dst nodes are degree-sorted into 98
    groups of 128 (one dst per partition). Each dst's incoming edges occupy
    padded slot columns; slot gathers use per-partition indirect DMA (128
    rows/call, int32 element offsets). Edge softmax without max-subtraction
    (numerically safe here); padding slots point at sentinel rows with
    el = -1e30 so exp() kills them. Weighted accumulation via fused DVE
    multiply-add over slot columns.
  - hp1 = h1 @ W1 (PE transpose + matmul), second 16-f32 table, AllGather,
    Phase E1 repeats the edge phase at width 7.
  - Host assembles the final [100000, 7] output (inverse degree-sort).
"""
import sys
sys.path.insert(0, "/opt/trn_rl_repo")
import numpy as np

N = 100000
NCORES = 8
SHARD = 12500
PSHARD = 12544          # 98 * 128
G = PSHARD // 128       # 98 groups
KDIM = 1536             # 1433 padded to 12*128
D0 = 140
D1 = 7
W0C = 144               # L0 table row: h(140), el(140), er(141), pad
W1C = 16                # L1 table row: hp1(7), el1(7), er1(8), pad
SENT = np.float32(-1e30)

_CACHE = {}
TRACE = False          # test harness sets this to capture an NTFF profile
LAST_EXEC_NS = None


def _host_prep(src, dst):
    src = np.asarray(src).astype(np.int64)
    dst = np.asarray(dst).astype(np.int64)
    deg = np.bincount(dst, minlength=N)
    nodes = np.arange(N, dtype=np.int64)
    pad_id = (nodes // SHARD) * PSHARD + (nodes % SHARD)  # original-order padded id

    percore = []
    Kg = np.zeros(G, dtype=np.int64)
    for c in range(NCORES):
        lo = c * SHARD
        m = (dst >= lo) & (dst < lo + SHARD)
        e_dst = dst[m] - lo
        e_src = src[m]
        d = deg[lo:lo + SHARD]
        order = np.argsort(-d, kind="stable")
        rank = np.empty(SHARD, dtype=np.int64)
        rank[order] = np.arange(SHARD)
        dsort = np.concatenate([d[order], np.zeros(PSHARD - SHARD, np.int64)])
        for g in range(G):
            Kg[g] = max(Kg[g], max(1, dsort[g * 128:(g + 1) * 128].max()))
        percore.append(dict(order=order, rank=rank, e_dst=e_dst, e_src=e_src))

    pos1 = np.empty(N, dtype=np.int64)  # sorted-space padded id
    for c in range(NCORES):
        pos1[c * SHARD:(c + 1) * SHARD] = c * PSHARD + percore[c]["rank"]

    SK = int(Kg.sum())
    cums = np.concatenate([[0], np.cumsum(Kg)])
    for c in range(NCORES):
        pc = percore[c]
        sent_row = c * PSHARD + PSHARD - 1
        r = pc["rank"][pc["e_dst"]]
        # self-loop edges first within each dst -> they land in slot 0
        not_self = (pc["e_src"] != pc["e_dst"] + c * SHARD).astype(np.int64)
        ordr = np.lexsort((not_self, r))
        r_s = r[ordr]
        kpos = np.arange(len(r_s)) - np.searchsorted(r_s, r_s)
        Kcap = int(Kg.max())
        slots0 = np.full((PSHARD, Kcap), sent_row, dtype=np.int64)
        slots0[r_s, kpos] = pad_id[pc["e_src"][ordr]]
        slots1 = np.full((PSHARD, Kcap), sent_row, dtype=np.int64)
        slots1[r_s, kpos] = pos1[pc["e_src"][ordr]]
        # pack per-group [128, Kg[g]] -> [128, SK] (element offsets)
        offs0 = np.zeros((128, SK), np.int32)
        offs1 = np.zeros((128, SK), np.int32)
        for g in range(G):
            offs0[:, cums[g]:cums[g + 1]] = slots0[g * 128:(g + 1) * 128, :Kg[g]]
            offs1[:, cums[g]:cums[g + 1]] = slots1[g * 128:(g + 1) * 128, :Kg[g]]
            # slot 0 is always own-core (self-loop or sentinel): make it
            # shard0-relative so the gather can run before the AllGather
            offs0[:, cums[g]] -= c * PSHARD
        pc["offs0"] = offs0
        pc["offs1"] = offs1
    return percore, Kg.astype(int), cums.astype(int)


def _prep_weights(inputs):
    X = np.asarray(inputs["X"], np.float32)
    W0 = np.asarray(inputs["W0"], np.float32)
    al0 = np.asarray(inputs["al0"], np.float32)
    ar0 = np.asarray(inputs["ar0"], np.float32)
    b0 = np.asarray(inputs["b0"], np.float32)
    W1 = np.asarray(inputs["W1"], np.float32)
    al1 = np.asarray(inputs["al1"], np.float32)
    ar1 = np.asarray(inputs["ar1"], np.float32)
    b1 = np.asarray(inputs["b1"], np.float32)

    # W0 rearranged: [128, 12*140]; W0r[kp, k*140+j] = W0[k*128+kp, j]
    W0p = np.zeros((KDIM, D0), np.float32)
    W0p[:1433] = W0
    W0r = np.ascontiguousarray(
        W0p.reshape(12, 128, D0).transpose(1, 0, 2).reshape(128, 12 * D0))
    W1a = np.zeros((128, D1), np.float32)
    W1a[:128] = W1[:128]
    W1b = np.zeros((128, D1), np.float32)
    W1b[:12] = W1[128:140]
    bc = lambda v, w: np.broadcast_to(np.asarray(v, np.float32)[None, :],
                                      (128, w)).copy()
    wal1 = W1 @ al1
    war1 = W1 @ ar1
    ident = np.eye(128, dtype=np.float32)
    sent_mask = np.zeros((128, 1), np.float32)
    sent_mask[SHARD - (G - 1) * 128:, 0] = SENT   # partitions 84.. are pads
    com = dict(W0r=W0r, W1a=W1a, W1b=W1b, sent_mask=sent_mask,
               al0b=bc(al0, D0), ar0b=bc(ar0, D0), b0b=bc(b0, D0),
               wal1b=bc(wal1, D0), war1b=bc(war1, D0), b1b=bc(b1, D1),
               ident=ident)

    # X tiles per core: xt[n, kp, k*128+nf] = X[lo + n*128+nf, k*128+kp]
    xts = []
    for c in range(NCORES):
        lo = c * SHARD
        Xp = np.zeros((PSHARD, KDIM), np.float32)
        Xp[:SHARD, :1433] = X[lo:lo + SHARD, :]
        xt = Xp.reshape(G, 128, 12, 128).transpose(0, 3, 2, 1).reshape(G, 128, 12 * 128)
        xt = np.concatenate([xt, np.zeros((100 - G, 128, 12 * 128), np.float32)])
        xt = np.ascontiguousarray(
            xt.reshape(25, 4, 128, 1536).transpose(0, 2, 1, 3).reshape(25, 128, 4 * 1536))
        xts.append(xt)
    return com, xts


def _build(Kg, cums):
    import concourse.bass as bass
    import concourse.tile as tile
    from concourse import bacc, mybir
    dt = mybir.dt
    op = mybir.AluOpType
    act = mybir.ActivationFunctionType

    SK = int(sum(Kg))
    nc = bacc.Bacc("TRN2", target_bir_lowering=False, debug=False,
                   num_devices=NCORES)
    t_x = nc.dram_tensor("x_up", [25, 128, 4 * 12 * 128], dt.float32, kind="ExternalInput")
    t_w0 = nc.dram_tensor("w0r", [128, 12 * D0], dt.float32, kind="ExternalInput")
    t_w1a = nc.dram_tensor("w1a", [128, D1], dt.float32, kind="ExternalInput")
    t_w1b = nc.dram_tensor("w1b", [128, D1], dt.float32, kind="ExternalInput")
    t_al0 = nc.dram_tensor("al0b", [128, D0], dt.float32, kind="ExternalInput")
    t_ar0 = nc.dram_tensor("ar0b", [128, D0], dt.float32, kind="ExternalInput")
    t_b0 = nc.dram_tensor("b0b", [128, D0], dt.float32, kind="ExternalInput")
    t_wal1 = nc.dram_tensor("wal1b", [128, D0], dt.float32, kind="ExternalInput")
    t_war1 = nc.dram_tensor("war1b", [128, D0], dt.float32, kind="ExternalInput")
    t_b1 = nc.dram_tensor("b1b", [128, D1], dt.float32, kind="ExternalInput")
    t_id = nc.dram_tensor("ident", [128, 128], dt.float32, kind="ExternalInput")
    t_of0 = nc.dram_tensor("offs0", [128, SK], dt.int32, kind="ExternalInput")
    t_of1 = nc.dram_tensor("offs1", [128, SK], dt.int32, kind="ExternalInput")
    t_sm = nc.dram_tensor("sent_mask", [128, 1], dt.float32, kind="ExternalInput")
    t_out = nc.dram_tensor("out_buf", [PSHARD, D1], dt.float32, kind="ExternalOutput")

    with tile.TileContext(nc) as tc:
        with tc.tile_pool(name="const", bufs=1) as cpool, \
             tc.tile_pool(name="xload", bufs=2) as xpool, \
             tc.tile_pool(name="hex", bufs=3) as hexpool, \
             tc.tile_pool(name="gath", bufs=4) as gpool, \
             tc.tile_pool(name="work", bufs=3) as wpool, \
             tc.tile_pool(name="small", bufs=4) as spool, \
             tc.tile_pool(name="psum", bufs=2, space="PSUM") as ppool, \
             tc.tile_pool(name="dram", bufs=1, space="DRAM") as dpool:

            # constants
            w0_sb = cpool.tile([128, 12 * D0], dt.float32)
            nc.sync.dma_start(w0_sb[:], t_w0[:])
            w1a_sb = cpool.tile([128, D1], dt.float32)
            nc.sync.dma_start(w1a_sb[:], t_w1a[:])
            w1b_sb = cpool.tile([128, D1], dt.float32)
            nc.sync.dma_start(w1b_sb[:], t_w1b[:])
            al0_sb = cpool.tile([128, D0], dt.float32)
            nc.sync.dma_start(al0_sb[:], t_al0[:])
            ar0_sb = cpool.tile([128, D0], dt.float32)
            nc.sync.dma_start(ar0_sb[:], t_ar0[:])
            b0_sb = cpool.tile([128, D0], dt.float32)
            nc.sync.dma_start(b0_sb[:], t_b0[:])
            wal1_sb = cpool.tile([128, D0], dt.float32)
            nc.sync.dma_start(wal1_sb[:], t_wal1[:])
            war1_sb = cpool.tile([128, D0], dt.float32)
            nc.sync.dma_start(war1_sb[:], t_war1[:])
            b1_sb = cpool.tile([128, D1], dt.float32)
            nc.sync.dma_start(b1_sb[:], t_b1[:])
            id_sb = cpool.tile([128, 128], dt.float32)
            nc.sync.dma_start(id_sb[:], t_id[:])
            of0_sb = cpool.tile([128, SK], dt.int32)
            nc.sync.dma_start(of0_sb[:], t_of0[:])
            of1_sb = cpool.tile([128, SK], dt.int32)
            nc.sync.dma_start(of1_sb[:], t_of1[:])
            sm_sb = cpool.tile([128, 1], dt.float32)
            nc.sync.dma_start(sm_sb[:], t_sm[:])
            er1_all = cpool.tile([128, G], dt.float32)

            shard0 = dpool.tile([PSHARD, W0C], dt.float32)
            table0 = dpool.tile([NCORES * PSHARD, W0C], dt.float32, addr_space="Shared")
            shard1 = dpool.tile([PSHARD, W1C], dt.float32)
            table1 = dpool.tile([NCORES * PSHARD, W1C], dt.float32, addr_space="Shared")

            # ---- Phase M: h = X @ W0, pack [h, el, er] rows ----
            XB = 4                      # X tiles per DMA (amortize fixed cost)
            xts = {}
            for n in range(G):
                b, t = n // XB, n % XB
                if t == 0:
                    xtile = xpool.tile([128, 4 * 12 * 128], dt.float32, tag="xt")
                    xts[b] = xtile
                    nc.sync.dma_start(xtile[:], t_x[:][b])
                xt = xts[b][:, t * 1536:(t + 1) * 1536]
                ph = ppool.tile([128, D0], dt.float32, space="PSUM")
                for k in range(12):
                    nc.tensor.matmul(ph[:], xt[:, k * 128:(k + 1) * 128],
                                     w0_sb[:, k * D0:(k + 1) * D0],
                                     start=(k == 0), stop=(k == 11))
                hx = hexpool.tile([128, W0C], dt.float32, tag="hex0")
                nc.vector.tensor_copy(hx[:, 0:D0], ph[:])
                scr = wpool.tile([128, D0], dt.float32, tag="mscr")
                nc.vector.tensor_tensor(scr[:], ph[:], al0_sb[:], op=op.mult)
                nc.vector.tensor_reduce(hx[:, 140:141], scr[:],
                                        axis=mybir.AxisListType.X, op=op.add)
                nc.vector.tensor_tensor(scr[:], ph[:], ar0_sb[:], op=op.mult)
                nc.vector.tensor_reduce(hx[:, 141:142], scr[:],
                                        axis=mybir.AxisListType.X, op=op.add)
                nc.vector.memset(hx[:, 142:144], 0.0)
                if n == G - 1:
                    nc.vector.tensor_tensor(hx[:, 140:141], hx[:, 140:141],
                                            sm_sb[:], op=op.add)
                nc.sync.dma_start(shard0[:].rearrange("(g p) w -> g p w", p=128)[n],
                                  hx[:])

            nc.gpsimd.collective_compute(
                "AllGather", op.bypass, replica_groups=[list(range(NCORES))],
                ins=[shard0[:]], outs=[table0[:]])

            # ---- Phase E0 ----
            for g in range(G):
                K = int(Kg[g])
                gt = gpool.tile([128, K * W0C], dt.float32, tag="g0")
                gv = gt[:].rearrange("p (k w) -> p k w", w=W0C)
                for k in range(K):
                    nc.gpsimd.indirect_dma_start(
                        out=gv[:, k], out_offset=None,
                        in_=shard0[:] if k == 0 else table0[:],
                        in_offset=bass.IndirectOffsetOnAxis(
                            ap=of0_sb[:, cums[g] + k:cums[g] + k + 1], axis=0))
                # slot 0 is the self-loop -> its row IS the dst row; er = col 141
                ep = spool.tile([128, K], dt.float32, tag="ep0")
                nc.vector.tensor_scalar(ep[:], gv[:, :, 140], gv[:, 0, 141:142],
                                        None, op.add)
                ee = spool.tile([128, K], dt.float32, tag="ee0")
                nc.vector.scalar_tensor_tensor(
                    out=ee[:], in0=ep[:], scalar=0.2, in1=ep[:],
                    op0=op.mult, op1=op.max)
                ex = spool.tile([128, K], dt.float32, tag="ex0")
                dn = spool.tile([128, 1], dt.float32, tag="dn0")
                nc.scalar.activation(ex[:], ee[:], act.Exp, accum_out=dn[:])
                nc.vector.tensor_scalar_max(dn[:], dn[:], 1e-30)
                rv = spool.tile([128, 1], dt.float32, tag="rv0")
                nc.vector.reciprocal(rv[:], dn[:])
                acc = wpool.tile([128, D0], dt.float32, tag="acc0")
                nc.vector.tensor_scalar(acc[:], gv[:, 0, 0:D0], ex[:, 0:1], None,
                                        op.mult)
                for k in range(1, K):
                    nc.vector.scalar_tensor_tensor(
                        out=acc[:], in0=gv[:, k, 0:D0], scalar=ex[:, k:k + 1],
                        in1=acc[:], op0=op.mult, op1=op.add)
                h1 = wpool.tile([128, D0], dt.float32, tag="h1")
                nc.vector.scalar_tensor_tensor(
                    out=h1[:], in0=acc[:], scalar=rv[:], in1=b0_sb[:],
                    op0=op.mult, op1=op.add)
                nc.scalar.activation(h1[:], h1[:], act.Relu)
                # el1/er1
                hx1 = hexpool.tile([128, W1C], dt.float32, tag="hex1")
                scr1 = wpool.tile([128, D0], dt.float32, tag="escr")
                nc.vector.tensor_tensor(scr1[:], h1[:], wal1_sb[:], op=op.mult)
                nc.vector.tensor_reduce(hx1[:, 7:8], scr1[:],
                                        axis=mybir.AxisListType.X, op=op.add)
                nc.vector.tensor_tensor(scr1[:], h1[:], war1_sb[:], op=op.mult)
                nc.vector.tensor_reduce(hx1[:, 8:9], scr1[:],
                                        axis=mybir.AxisListType.X, op=op.add)
                nc.vector.tensor_copy(er1_all[:, g:g + 1], hx1[:, 8:9])
                # hp1 = h1 @ W1 via PE transpose
                pt1 = ppool.tile([128, 128], dt.float32, space="PSUM", tag="pt1")
                nc.tensor.transpose(pt1[:], h1[:, 0:128], id_sb[:])
                pt2 = ppool.tile([128, 128], dt.float32, space="PSUM", tag="pt2")
                nc.tensor.transpose(pt2[0:12, :], h1[:, 128:140], id_sb[:])
                t1s = wpool.tile([128, 128], dt.float32, tag="t1s")
                nc.vector.tensor_copy(t1s[:], pt1[:])
                t2s = wpool.tile([128, 128], dt.float32, tag="t2s")
                nc.vector.tensor_copy(t2s[0:12, :], pt2[0:12, :])
                php = ppool.tile([128, D1], dt.float32, space="PSUM", tag="php")
                nc.tensor.matmul(php[:], t1s[:], w1a_sb[:], start=True, stop=False)
                nc.tensor.matmul(php[:], t2s[0:12, :], w1b_sb[0:12, :],
                                 start=False, stop=True)
                nc.vector.tensor_copy(hx1[:, 0:D1], php[:])
                nc.vector.memset(hx1[:, 9:16], 0.0)
                if g == G - 1:
                    nc.vector.tensor_tensor(hx1[:, 7:8], hx1[:, 7:8],
                                            sm_sb[:], op=op.add)
                nc.sync.dma_start(shard1[:].rearrange("(g p) w -> g p w", p=128)[g],
                                  hx1[:])

            nc.gpsimd.collective_compute(
                "AllGather", op.bypass, replica_groups=[list(range(NCORES))],
                ins=[shard1[:]], outs=[table1[:]])

            # ---- Phase E1 ----
            for g in range(G):
                K = int(Kg[g])
                gt = gpool.tile([128, K * W1C], dt.float32, tag="g1")
                gv = gt[:].rearrange("p (k w) -> p k w", w=W1C)
                # slot 0 = self-loop: contiguous rows of our own shard (sorted space)
                nc.sync.dma_start(
                    gv[:, 0], shard1[:].rearrange("(g p) w -> g p w", p=128)[g])
                for k in range(1, K):
                    nc.gpsimd.indirect_dma_start(
                        out=gv[:, k], out_offset=None, in_=table1[:],
                        in_offset=bass.IndirectOffsetOnAxis(
                            ap=of1_sb[:, cums[g] + k:cums[g] + k + 1], axis=0))
                ep = spool.tile([128, K], dt.float32, tag="ep1")
                nc.vector.tensor_scalar(ep[:], gv[:, :, 7], er1_all[:, g:g + 1],
                                        None, op.add)
                ee = spool.tile([128, K], dt.float32, tag="ee1")
                nc.vector.scalar_tensor_tensor(
                    out=ee[:], in0=ep[:], scalar=0.2, in1=ep[:],
                    op0=op.mult, op1=op.max)
                ex = spool.tile([128, K], dt.float32, tag="ex1")
                dn = spool.tile([128, 1], dt.float32, tag="dn1")
                nc.scalar.activation(ex[:], ee[:], act.Exp, accum_out=dn[:])
                nc.vector.tensor_scalar_max(dn[:], dn[:], 1e-30)
                rv = spool.tile([128, 1], dt.float32, tag="rv1")
                nc.vector.reciprocal(rv[:], dn[:])
                acc = spool.tile([128, D1], dt.float32, tag="acc1")
                nc.vector.tensor_scalar(acc[:], gv[:, 0, 0:D1], ex[:, 0:1], None,
                                        op.mult)
                for k in range(1, K):
                    nc.vector.scalar_tensor_tensor(
                        out=acc[:], in0=gv[:, k, 0:D1], scalar=ex[:, k:k + 1],
                        in1=acc[:], op0=op.mult, op1=op.add)
                ot = spool.tile([128, D1], dt.float32, tag="ot")
                nc.vector.scalar_tensor_tensor(
                    out=ot[:], in0=acc[:], scalar=rv[:], in1=b1_sb[:],
                    op0=op.mult, op1=op.add)
                nc.scalar.activation(ot[:], ot[:], act.Relu)
                nc.sync.dma_start(t_out[:].rearrange("(g p) w -> g p w", p=128)[g],
                                  ot[:])
    nc.compile()
    return nc


def kernel(**inputs):
    percore, Kg, cums = _host_prep(inputs["src"], inputs["dst"])
    com, xts = _prep_weights(inputs)

    key = tuple(Kg)
    if key not in _CACHE:
        _CACHE[key] = _build(Kg, cums)
    nc = _CACHE[key]

    in_maps = []
    for c in range(NCORES):
        pc = percore[c]
        m = dict(x_up=xts[c], w0r=com["W0r"], w1a=com["W1a"], w1b=com["W1b"],
                 al0b=com["al0b"], ar0b=com["ar0b"], b0b=com["b0b"],
                 wal1b=com["wal1b"], war1b=com["war1b"], b1b=com["b1b"],
                 ident=com["ident"], offs0=pc["offs0"], offs1=pc["offs1"],
                 sent_mask=com["sent_mask"])
        in_maps.append(m)

    from concourse.bass_utils import run_bass_kernel_spmd
    global LAST_EXEC_NS
    res = run_bass_kernel_spmd(nc, in_maps, core_ids=list(range(NCORES)),
                               trace=TRACE)
    LAST_EXEC_NS = res.exec_time_ns
    out = np.zeros((N, D1), dtype=np.float32)
    for c in range(NCORES):
        ob = res.results[c]["out_buf"]
        out[c * SHARD + percore[c]["order"]] = ob[:SHARD]
    return out

